# revision 19
# baseline (speedup 1.0000x reference)
"""Trainium2 Bass kernel for nn_ABNet: 10-head MLP + CBF-QP projection.

Data-parallel over 8 NeuronCores: batch 32768 -> 4096 rows/core, weights
replicated. Structure (per core):
  - Mid-layer matmuls in fp8e4m3 + DoubleRow on the TensorEngine,
    activations in [feat, batch] layout, processed in two batch halves.
  - A software-pipelined wave over (half, head) with per-layer head
    lags: evac-heavy L1 units interleave with PE-heavy mid units so
    ScalarE/VectorE (PSUM evacuation) and the PE stay busy together.
  - PSUM evacuation (relu+bias+fp8 cast) is width-balanced across the
    two PSUM-capable lanes: ScalarE activation and VectorE
    tensor_scalar.
  - L5 is column-group packed: 4 heads' fp8+DoubleRow matmuls run
    concurrently in the 4 PE column groups; one [128,512] copy
    evacuates 4 heads at once.
  - The CBF geometry + QP halfspace projection runs per half in fp32
    on Vector/GpSimd engines in batch-major layout, overlapping the
    other half's matmul wave.
"""

import sys

for _p in ("/opt/trn_rl_repo", "/root/.axon_site/_ro/trn_rl_repo"):
    if _p not in sys.path:
        sys.path.append(_p)

import numpy as np
import ml_dtypes

import concourse.bass as bass
import concourse.mybir as mybir
from concourse.tile import TileContext
from concourse.masks import make_identity

H = 10
B = 32768
NCORES = 8
BC = B // NCORES          # batch per core = 4096
NHALF = 1                 # batch halves per core (full batch per wave)
BH = BC // NHALF          # 4096
CH = 512                  # matmul moving chunk (one PSUM bank)
NT = BC // 128            # 128-wide batch tiles = 32
NTH = BH // 128           # per half = 16
D = 256
L1C, L2C = 3.0, 3.0
OBS_X, OBS_Y, R = 0.0, 7.0, 4.0
L5SC = 8.0                # host-side scale on L5 weights (fp8 range fix)

F32 = mybir.dt.float32
BF16 = mybir.dt.bfloat16
F8 = mybir.dt.float8e4
DR = mybir.MatmulPerfMode.DoubleRow
AF = mybir.ActivationFunctionType
OP = mybir.AluOpType

MAX_WAITS = 1

# evac lane shares (ScalarE, VectorE) — only these two engines can read
# PSUM. ACT psum op ~(172+FD)/1.2 ns vs DVE ~(120+FD)/0.96: 0.54/0.46.
EVAC_W = {"act": 0.54, "dve": 0.46}


def _split_waits(nc):
    """walrus in this env rejects >1 sync-wait per instruction; split extras
    onto preceding same-engine NOPs (semantically identical: sequential
    waits on an in-order engine)."""
    fn = nc.m.functions[0]
    for bb in fn.blocks:
        new = []
        for inst in bb.instructions:
            si = getattr(inst, "sync_info", None)
            if si is not None and si.on_wait and len(si.on_wait) > MAX_WAITS:
                waits = list(si.on_wait)
                while len(waits) > MAX_WAITS:
                    chunk, waits = waits[:MAX_WAITS], waits[MAX_WAITS:]
                    new.append(
                        mybir.InstNoOp(
                            name=nc.get_next_instruction_name(),
                            engine=inst.engine,
                            sync_info=mybir.SyncInfo(on_wait=chunk, on_update=[]),
                            bass_nofuse=True,
                        )
                    )
                si.on_wait = waits
            new.append(inst)
        bb.instructions[:] = new


def _dedup_ldweights(nc):
    """Drop InstLdweights whose payload matches the weights already loaded
    in that PE quadrant (the Tile scheduler emits one per matmul; the PE
    weight regs persist, and the paired matmuls carry ldweights=False).
    Full-width loads (>32 rows) clobber every quadrant. Sync carried by a
    dropped load moves onto a NOP. Validated numerically on hardware."""
    fn = nc.m.functions[0]
    removed = 0
    for bb in fn.blocks:
        tracker = {}
        new = []
        for inst in bb.instructions:
            if type(inst).__name__ == "InstLdweights":
                ts = inst.tile_size or (128, 128)
                key = tuple(inst.tile_position or (0, 0))
                a0 = inst.ins[0]
                payload = (str(a0.dtype), str(a0.ap), a0.offset,
                           str(getattr(a0, "memref", None)),
                           str(inst.perf_mode), str(inst.tile_size),
                           str(inst.is_transpose))
                if ts[0] > 32:
                    for k in [k for k in tracker if k != key]:
                        del tracker[k]
                if tracker.get(key) == payload:
                    si = inst.sync_info
                    if si is not None and (si.on_wait or si.on_update):
                        new.append(mybir.InstNoOp(
                            name=nc.get_next_instruction_name(),
                            engine=inst.engine, sync_info=si,
                            bass_nofuse=True))
                    removed += 1
                    continue
                tracker[key] = payload
            new.append(inst)
        bb.instructions[:] = new
    return removed


def _bcast(t_ap, n=128):
    """Partition-broadcast AP for a DRAM tensor (step-0 partition dim)."""
    return bass.AP(tensor=t_ap.tensor, offset=t_ap.offset, ap=[[0, n]] + list(t_ap.ap))


def build_graph():
    nc = bass.Bass()
    dp = nc.declare_dram_parameter

    xT = dp("xT", [4, BC], BF16, isOutput=False)
    xg = dp("xg", [128, NT, 4], F32, isOutput=False)
    w1t = dp("w1t", [4, H * D], BF16, isOutput=False)
    # all 5 mid-layer weights for one head in a single DMA-able block
    whead = dp("whead", [H, 128, 5, 2, D], F8, isOutput=False)
    bmid = {
        name: dp(name, [128, H, 2], F32, isOutput=False)
        for name in ("b1", "b2", "b31", "b32", "b41", "b42")
    }
    # L5 weights: [p, h, branch, kt, col(16 padded)], fp8, scaled by L5SC
    l5w = dp("l5w", [128, H, 2, 2, 32], F8, isOutput=False)
    b51v = dp("b51v", [1, 2 * H], F32, isOutput=False)
    b52v = dp("b52v", [1, 2 * H], F32, isOutput=False)
    wt = dp("wt", [1, H], F32, isOutput=False)
    mlab = dp("mlab", [1, 2], F32, isOutput=False)
    slab = dp("slab", [1, 2], F32, isOutput=False)
    meanp = dp("meanp", [1, 4], F32, isOutput=False)
    stdp = dp("stdp", [1, 4], F32, isOutput=False)
    out = dp("out", [BC, 2], F32, isOutput=True)

    wsm_dram = nc.dram_tensor("wsm_dram", [1, H], F32)

    with TileContext(nc) as tc:
        wpool = tc.alloc_tile_pool(name="weights", bufs=1)
        apool = tc.alloc_tile_pool(name="acts", bufs=1)
        qpool = tc.alloc_tile_pool(name="qp", bufs=1)
        gpool = tc.alloc_tile_pool(name="stg", bufs=1)
        ppool = tc.alloc_tile_pool(name="psum", bufs=4, space="PSUM")

        # ---- loads ------------------------------------------------------
        # bulk loads via gpsimd (SWDGE, 8 queues); consumption-order.
        # L1 operands are replicated at partition bases 0/32/64/96 for
        # row-group-packed (4x concurrent) K=4 matmuls.
        xT4_sb = wpool.tile([128, BC], BF16, name="xT4")
        w14_sb = wpool.tile([128, H * D], BF16, name="w14")
        for g in range(4):
            nc.gpsimd.dma_start(out=xT4_sb[32 * g : 32 * g + 4, :], in_=xT[:, :])
            nc.gpsimd.dma_start(out=w14_sb[32 * g : 32 * g + 4, :], in_=w1t[:, :])
        bmid_sb = {}
        whead_sb = [None] * H

        def load_whead(h):
            wtile = wpool.tile([128, 5, 2, D], F8, name=f"whead_{h}")
            nc.gpsimd.dma_start(out=wtile, in_=whead[h])
            whead_sb[h] = wtile

        def load_bias(name):
            btile = wpool.tile([128, H, 2], F32, name=f"{name}_sb")
            nc.gpsimd.dma_start(out=btile, in_=bmid[name][:, :, :])
            bmid_sb[name] = btile

        load_bias("b1")
        xg_sb = qpool.tile([128, NT, 4], F32)
        nc.gpsimd.dma_start(out=xg_sb, in_=xg[:, :, :])

        def bc_tile(src_ap, n, name):
            t = qpool.tile([128, n], F32, name=name)
            nc.gpsimd.dma_start(out=t, in_=_bcast(src_ap))
            return t

        b51_bc = bc_tile(b51v[0], 2 * H, "b51bc")
        b52_bc = bc_tile(b52v[0], 2 * H, "b52bc")
        mlab_bc = bc_tile(mlab[0], 2, "mlabbc")
        slab_bc = bc_tile(slab[0], 2, "slabbc")
        mean_bc = bc_tile(meanp[0], 4, "meanbc")
        std_bc = bc_tile(stdp[0], 4, "stdbc")

        load_bias("b2")
        for h in range(H):
            load_whead(h)
        for name in ("b31", "b32", "b41", "b42"):
            load_bias(name)
        l5w_sb = wpool.tile([128, H, 2, 2, 32], F8, name="l5w_sb")
        nc.gpsimd.dma_start(out=l5w_sb, in_=l5w[:, :, :, :, :])

        ident = wpool.tile([128, 128], F32, name="ident")
        make_identity(nc, ident[:, :])

        # ---- softmax(wt) -------------------------------------------------
        wt_sb = qpool.tile([1, H], F32, name="wt_sb")
        nc.sync.dma_start(out=wt_sb, in_=wt[:, :])
        wexp = qpool.tile([1, H], F32, name="wexp")
        zeroh = qpool.tile([1, 1], F32, name="zeroh")
        nc.vector.memset(zeroh, 0.0)
        nc.scalar.activation(wexp, wt_sb, AF.Exp, bias=zeroh[:, 0:1])
        wsum = qpool.tile([1, 1], F32, name="wsum")
        nc.vector.reduce_sum(out=wsum, in_=wexp, axis=mybir.AxisListType.X)
        winv = qpool.tile([1, 1], F32, name="winv")
        nc.vector.reciprocal(winv, wsum)
        wnorm = qpool.tile([1, H], F32, name="wnorm")
        nc.vector.tensor_scalar(wnorm, wexp, winv[:, 0:1], None, OP.mult)
        nc.sync.dma_start(out=wsm_dram[:, :], in_=wnorm)
        w_bc = bc_tile(wsm_dram[0], H, "wbc")

        # ---- geometry (batch-major [128, NT] fp32, full width) -----------
        def qt(name):
            return qpool.tile([128, NT], F32, name=name)

        tt = nc.vector.tensor_tensor
        ts = nc.vector.tensor_scalar
        stt = nc.vector.scalar_tensor_tensor

        t1q, w1q, t2q, w2q = qt("t1q"), qt("w1q"), qt("t2q"), qt("w2q")
        for dst, f in ((t1q, 0), (w1q, 1), (t2q, 2), (w2q, 3)):
            ts(dst, xg_sb[:, :, f], std_bc[:, f : f + 1], mean_bc[:, f : f + 1],
               OP.mult, OP.add)

        s1, c1, s2, c2 = qt("s1"), qt("c1"), qt("s2"), qt("c2")
        zero1 = qpool.tile([128, 1], F32, name="zero1")
        nc.vector.memset(zero1, 0.0)
        pi2 = qpool.tile([128, 1], F32, name="pi2")
        nc.vector.memset(pi2, float(np.pi / 2))
        nc.scalar.activation(s1, t1q, AF.Sin, bias=zero1[:, 0:1])
        nc.scalar.activation(c1, t1q, AF.Sin, bias=pi2[:, 0:1])
        nc.scalar.activation(s2, t2q, AF.Sin, bias=zero1[:, 0:1])
        nc.scalar.activation(c2, t2q, AF.Sin, bias=pi2[:, 0:1])

        px, py, vx, vy = qt("px"), qt("py"), qt("vx"), qt("vy")
        tmp1, tmp2, tmp3 = qt("tmp1"), qt("tmp2"), qt("tmp3")
        tt(tmp1, c1, c2, OP.add)
        ts(px, tmp1, L1C, -OBS_X, OP.mult, OP.add)
        tt(tmp1, s1, s2, OP.add)
        ts(py, tmp1, L1C, -OBS_Y, OP.mult, OP.add)
        tt(tmp1, s1, w1q, OP.mult)
        tt(tmp2, s2, w2q, OP.mult)
        tt(tmp3, tmp1, tmp2, OP.add)
        ts(vx, tmp3, -L1C, None, OP.mult)
        tt(tmp1, c1, w1q, OP.mult)
        tt(tmp2, c2, w2q, OP.mult)
        tt(tmp3, tmp1, tmp2, OP.add)
        ts(vy, tmp3, L1C, None, OP.mult)

        bar16, bdot4, lf2b = qt("bar16"), qt("bdot4"), qt("lf2b")
        tt(tmp1, px, px, OP.mult)
        tt(tmp2, py, py, OP.mult)
        tt(tmp3, tmp1, tmp2, OP.add)
        ts(bar16, tmp3, 16.0, -16.0 * R * R, OP.mult, OP.add)
        tt(tmp1, px, vx, OP.mult)
        tt(tmp2, py, vy, OP.mult)
        tt(tmp3, tmp1, tmp2, OP.add)
        ts(bdot4, tmp3, 8.0, None, OP.mult)

        w1sq, w2sq = qt("w1sq"), qt("w2sq")
        tt(w1sq, w1q, w1q, OP.mult)
        tt(w2sq, w2q, w2q, OP.mult)
        tt(tmp1, c1, w1sq, OP.mult)
        tt(tmp2, c2, w2sq, OP.mult)
        tt(tmp3, tmp1, tmp2, OP.add)
        ua = qt("ua")
        tt(ua, px, tmp3, OP.mult)
        tt(tmp1, s1, w1sq, OP.mult)
        tt(tmp2, s2, w2sq, OP.mult)
        tt(tmp3, tmp1, tmp2, OP.add)
        ub = qt("ub")
        tt(ub, py, tmp3, OP.mult)
        tt(tmp1, ua, ub, OP.add)
        tt(tmp2, vx, vx, OP.mult)
        tt(tmp3, vy, vy, OP.mult)
        ud = qt("ud")
        tt(ud, tmp2, tmp3, OP.add)
        ts(tmp2, tmp1, -6.0, None, OP.mult)
        stt(lf2b, ud, 2.0, tmp2, OP.mult, OP.add)

        g1, g2, igg = qt("g1"), qt("g2"), qt("igg")
        tt(tmp1, px, s1, OP.mult)
        tt(tmp2, py, c1, OP.mult)
        tt(tmp3, tmp1, tmp2, OP.subtract)
        ts(g1, tmp3, 2.0 * L1C, None, OP.mult)
        tt(tmp1, px, s2, OP.mult)
        tt(tmp2, py, c2, OP.mult)
        tt(tmp3, tmp1, tmp2, OP.subtract)
        ts(g2, tmp3, 2.0 * L2C, None, OP.mult)
        tt(tmp1, g1, g1, OP.mult)
        tt(tmp2, g2, g2, OP.mult)
        tt(tmp3, tmp1, tmp2, OP.add)
        nc.vector.reciprocal(igg, tmp3)

        isl = qpool.tile([128, 2], F32, name="isl")
        nc.vector.reciprocal(isl, slab_bc)

        # ---- evac router -------------------------------------------------
        # width-weighted round-robin over the 2 PSUM-capable lanes
        evac_n = {"act": 0, "dve": 0}
        evac_total = [0]

        def evac(dst_ap, psum_ap, bias_ap, width):
            """relu(psum + bias) -> dst (or plain copy when bias is None)
            across the 2 PSUM-capable lanes."""
            evac_total[0] += width
            lane = max(
                EVAC_W,
                key=lambda k: EVAC_W[k] * evac_total[0] - evac_n[k],
            )
            evac_n[lane] += width
            if lane == "act":
                if bias_ap is None:
                    nc.scalar.activation(dst_ap, psum_ap, AF.Copy)
                else:
                    nc.scalar.activation(dst_ap, psum_ap, AF.Relu,
                                         bias=bias_ap)
            else:
                if bias_ap is None:
                    nc.vector.tensor_copy(dst_ap, psum_ap)
                else:
                    nc.vector.tensor_scalar(dst_ap, psum_ap, bias_ap, 0.0,
                                            OP.add, OP.max)

        # ---- activation slot tiles (liveness-shared, 3 per head) --------
        # slotA: x1 [L1..L2], x31 [L31..L41], x42 [L42..L5]
        # slotB: x2 [L2..L32], x41 [L41..L5]
        # slotC: x32 [L32..L42]
        # Tags rotate across heads (h mod K) so only the wave's live
        # window of heads holds SBUF: slotA/B live <=5 wave steps, slotC
        # 3. Each .tile() call on a tag is a new generation in the same
        # memory (WAR-synced by Tile). `acts[h]` maps live value name ->
        # tile object so readers see the written tile.
        acts = {h: {} for h in range(H)}
        KROT = {"A": 5, "B": 5, "C": 3}

        def new_slot(h, s, vname):
            t = apool.tile([128, 2, BH], F8, tag=f"s{s}{h % KROT[s]}",
                           name=f"{vname}_{h}", bufs=1)
            acts[h][vname] = t
            return t

        # mid layers: (whead li idx, bias, src value, dst value, dst slot)
        MID = ((0, "b2", "x1", "x2", "B"), (1, "b31", "x2", "x31", "A"),
               (2, "b32", "x2", "x32", "C"), (3, "b41", "x31", "x41", "B"),
               (4, "b42", "x32", "x42", "A"))

        def l1_block(h, hb):
            # L1 (K=4): row-group packed, 4 concurrent matmuls filling
            # two 2-bank psum tiles, evacuated with wide ops.
            # Each row-group quadrant keeps its weights across cg
            # iterations, so cg=1's matmuls skip the reload.
            a1 = new_slot(h, "A", "x1")
            for cg in range(BH // (2 * CH)):
                psA = ppool.tile([128, 2 * CH], F32, tag="mm")
                psB = ppool.tile([128, 2 * CH], F32, tag="mm")
                for j, (mt, dc) in enumerate(
                    ((0, 0), (0, 1), (1, 0), (1, 1))
                ):
                    c = 2 * cg + dc
                    ps = psA if mt == 0 else psB
                    mmi = nc.tensor.matmul(
                        ps[:, dc * CH : (dc + 1) * CH],
                        w14_sb[32 * j : 32 * j + 4,
                               h * D + mt * 128 : h * D + (mt + 1) * 128],
                        xT4_sb[32 * j : 32 * j + 4,
                               hb + c * CH : hb + (c + 1) * CH],
                        start=True, stop=True,
                        tile_position=(32 * j, 0),
                    )
                    if cg > 0:
                        mmi.ins.ldweights = False
                csl = slice(2 * cg * CH, (2 * cg + 2) * CH)
                evac(a1[:, 0, csl], psA[:, :], bmid_sb["b1"][:, h, 0:1], 1024)
                evac(a1[:, 1, csl], psB[:, :], bmid_sb["b1"][:, h, 1:2], 1024)

        def mid_block(li, h):
            wli, bname, srcv, dstv, dsts = MID[li]
            src_t = acts[h][srcv]
            dst_t = new_slot(h, dsts, dstv)
            wtiles = whead_sb[h]
            btile = bmid_sb[bname]
            # mt-outer: 4 consecutive matmuls share the stationary weights;
            # only the first issues LDWEIGHTS (weight port was the
            # bottleneck: DR loads are 256 cols ~213ns each)
            for mt in range(2):
                for cg in range(BH // (2 * CH)):
                    ps = ppool.tile([128, 2 * CH], F32, tag="mm")
                    for dc in range(2):
                        c = 2 * cg + dc
                        mmi = nc.tensor.matmul(
                            ps[:, dc * CH : (dc + 1) * CH],
                            wtiles[:, wli, :, mt * 128 : (mt + 1) * 128],
                            src_t[:, :, c * CH : (c + 1) * CH],
                            start=True, stop=True,
                            perf_mode=DR,
                        )
                        if not (cg == 0 and dc == 0):
                            mmi.ins.ldweights = False
                    csl = slice(2 * cg * CH, (2 * cg + 2) * CH)
                    evac(dst_t[:, mt, csl], ps[:, :], btile[:, h, mt : mt + 1],
                         1024)

        x5T = qpool.tile([128, NT, 4 * H], F32, name="x5T")
        out_r = out.rearrange("(t p) c -> p t c", p=128)

        # interleaved [g1|g2] pairs for 2-wide dot products (shared)
        g12 = qpool.tile([128, NT, 2], F32, name="g12")
        nc.gpsimd.tensor_copy(g12[:, :, 0], g1)
        nc.gpsimd.tensor_copy(g12[:, :, 1], g2)
        zq = qt("zq")
        nc.gpsimd.memset(zq, 0.0)
        gt = nc.gpsimd.tensor_tensor

        def l5_group(heads, half, x5all):
            """L5 for a pair of heads, col-group packed: head i's fp8
            matmuls land in PE column group i (psum partitions 32i..),
            so the two heads' streams run concurrently. DoubleRow demands
            dst partition 0 (s3d3 ISA check), so this runs normal-mode
            fp8: K=128 per matmul, (branch, kt) accumulate. Two moving
            chunks share one borrowed psum generation; one [64, 2*CH]
            engine copy evacuates both heads, then per-head DMAs gather
            the 4 real rows into x5all."""
            for cp in range(BH // (2 * CH)):
                ps5 = ppool.tile([128, 2 * CH], F32, tag="mm")
                for b in range(2):
                    for kt in range(2):
                        for gi, h in enumerate(heads):
                            src = acts[h]["x41" if b == 0 else "x42"]
                            for dc in range(2):
                                c = 2 * cp + dc
                                nc.tensor.matmul(
                                    ps5[32 * gi : 32 * gi + 32,
                                        dc * CH : (dc + 1) * CH],
                                    l5w_sb[:, h, b, kt, :],
                                    src[:, kt, c * CH : (c + 1) * CH],
                                    start=(b == 0 and kt == 0),
                                    stop=(b == 1 and kt == 1),
                                    tile_position=(0, 32 * gi),
                                )
                stg5 = gpool.tile([64, 2 * CH], F32, tag="t_stg5", bufs=2)
                evac(stg5, ps5[0:64, :], None, 2 * CH)
                for gi, h in enumerate(heads):
                    for dc in range(2):
                        c = 2 * cp + dc
                        nc.sync.dma_start(
                            out=x5all[4 * h : 4 * h + 4,
                                      c * CH : (c + 1) * CH],
                            in_=stg5[32 * gi : 32 * gi + 4,
                                     dc * CH : (dc + 1) * CH],
                        )

        def qp_half(half):
            """QP projection for one half ([128, NTH] wide). Pool does the
            per-head geometry chain; DVE does relu + weight-accumulate."""
            hsl = slice(half * NTH, (half + 1) * NTH)
            x5Th = x5T[:, hsl, :]
            x5v = x5Th.rearrange("p t (h q) -> p t h q", q=4)
            u_view = x5v[:, :, :, 0:2]
            s_view = x5v[:, :, :, 2:4]

            def nt_bcast(v):
                return bass.AP(tensor=v.tensor, offset=v.offset,
                               ap=[list(v.ap[0]), [0, NTH], [2, H], [1, 2]])

            # x5 carries the L5SC weight scale; fold 1/L5SC here
            stt(u_view, u_view, 1.0 / L5SC, nt_bcast(b51_bc[:, :]),
                OP.mult, OP.subtract)
            stt(s_view, s_view, 1.0 / L5SC, nt_bcast(b52_bc[:, :]),
                OP.mult, OP.add)
            nc.scalar.activation(s_view, s_view, AF.Sigmoid,
                                 bias=zero1[:, 0:1])

            def qth(name):
                return qpool.tile([128, NTH], F32, name=f"{name}_{half}")

            def qth3(name):
                return qpool.tile([128, NTH, H], F32, name=f"{name}_{half}")

            def h_bc(v):
                # [128, NTH] AP -> [128, NTH, H] (0-step head dim)
                return bass.AP(tensor=v.tensor, offset=v.offset,
                               ap=[list(v.ap[0]), list(v.ap[1]), [0, H]])

            def w_bc3(v):
                # [128, H] AP -> [128, NTH, H] (0-step tile dim)
                return bass.AP(tensor=v.tensor, offset=v.offset,
                               ap=[list(v.ap[0]), [0, NTH], list(v.ap[1])])

            s0 = x5Th[:, :, 2]
            Aq, Dq = qth("Aq"), qth("Dq")
            htmp1, htmp2 = qth("htmp1"), qth("htmp2")
            gt(htmp1, s0, bdot4[:, hsl], OP.mult)
            gt(Aq, lf2b[:, hsl], htmp1, OP.add)
            gt(htmp1, s0, bar16[:, hsl], OP.mult)
            gt(Dq, bdot4[:, hsl], htmp1, OP.add)

            # batched across heads: [128, NTH, H] ops (head is a free
            # dim; per-sample geometry broadcast via 0-step APs) so the
            # per-op fixed cost amortizes 10x vs a per-head loop.
            u1 = x5v[:, :, :, 0]
            u2 = x5v[:, :, :, 1]
            sb = x5v[:, :, :, 3]
            g1h = g12[:, hsl, 0]
            g2h = g12[:, hsl, 1]
            wv = w_bc[:, 0:H]

            m1, m2 = qth3("m1"), qth3("m2")
            q1, e3 = qth3("q1"), qth3("e3")
            tt(m1, u1, h_bc(g1h), OP.mult)
            tt(m2, u2, h_bc(g2h), OP.mult)
            tt(q1, sb, h_bc(Dq[:, :]), OP.mult)
            gt(m1, m1, m2, OP.add)                      # s12
            tt(q1, q1, h_bc(Aq[:, :]), OP.add)          # hq
            gt(m1, m1, q1, OP.subtract)                 # viol
            ts(e3, m1, 0.0, None, OP.max)
            tt(e3, e3, w_bc3(wv), OP.mult)
            acc_e = qth("acc_e")
            nc.vector.reduce_sum(out=acc_e, in_=e3, axis=mybir.AxisListType.X)
            acc_u12 = qpool.tile([128, NTH, 2], F32, name=f"acc_u12_{half}")
            tt(m2, u1, w_bc3(wv), OP.mult)
            nc.vector.reduce_sum(out=acc_u12[:, :, 0], in_=m2,
                                 axis=mybir.AxisListType.X)
            tt(m2, u2, w_bc3(wv), OP.mult)
            nc.vector.reduce_sum(out=acc_u12[:, :, 1], in_=m2,
                                 axis=mybir.AxisListType.X)

            lamw = qth("lamw")
            tt(lamw, acc_e, igg[:, hsl], OP.mult)
            out_t = qpool.tile([128, NTH, 2], F32, name=f"out_t_{half}")
            g12h = g12[:, hsl, :]
            for cix in range(2):
                g_c = g12h[:, :, cix]
                tt(htmp1, lamw, g_c, OP.mult)
                tt(htmp2, acc_u12[:, :, cix], htmp1, OP.subtract)
                ts(out_t[:, :, cix], htmp2, mlab_bc[:, cix : cix + 1],
                   isl[:, cix : cix + 1], OP.subtract, OP.mult)
            nc.sync.dma_start(out=out_r[:, hsl, :], in_=out_t)

        # ================= software-pipelined wave =======================
        # One global stream over g = half*H + head with per-layer head
        # lags, so evac-heavy L1 units blend with the PE-heavy mid sweeps
        # and ACT/DVE/PE stay simultaneously busy. Unit (layer, g) is
        # emitted at step s = g + lag; L5 col-groups, the transposes and
        # the QP chain for a half enter the stream as soon as their last
        # producer unit has been emitted.
        x5alls = [
            gpool.tile([40, BH], F32, tag="x5all", bufs=NHALF,
                       name=f"x5all_{hf}")
            for hf in range(NHALF)
        ]
        G = NHALF * H
        L5_GROUPS = [[2 * i, 2 * i + 1] for i in range(H // 2)]

        def transpose_half(half):
            # x5all [40, BH] -> x5T [128, NT, 40]; transpose psum borrows
            # ppool "mm" tiles (subrange) to keep the psum budget at 8
            # banks.
            for t in range(NTH):
                pst_full = ppool.tile([128, 2 * CH], F32, tag="mm",
                                      name=f"trp_{half}_{t}")
                pst = pst_full[:, 0 : 4 * H]
                nc.tensor.transpose(
                    pst, x5alls[half][:, t * 128 : (t + 1) * 128],
                    ident[0 : 4 * H, 0 : 4 * H],
                )
                tglob = half * NTH + t
                if t % 2 == 0:
                    nc.vector.tensor_copy(x5T[:, tglob, :], pst)
                else:
                    nc.scalar.activation(x5T[:, tglob, :], pst, AF.Copy)

        MID_LAG = (1, 2, 2, 3, 3)
        for s in range(G + 4):
            if s < G:
                half, h = divmod(s, H)
                l1_block(h, half * BH)
            for li, lag in enumerate(MID_LAG):
                g = s - lag
                if 0 <= g < G:
                    half, h = divmod(g, H)
                    mid_block(li, h)
            # L5 col-groups fire right after their heads' L42 units
            g = s - 3
            if 0 <= g < G:
                half, h = divmod(g, H)
                for grp in L5_GROUPS:
                    if grp[-1] == h:
                        l5_group(grp, half, x5alls[half])
                        if h == H - 1:
                            transpose_half(half)
                            qp_half(half)

        for pool in (ppool, gpool, qpool, apool, wpool):
            pool.release()

    _dedup_ldweights(nc)
    _split_waits(nc)
    return nc


def prep_inputs(inputs):
    """Host-side shard + layout prep. Returns in_maps for 8 cores."""
    f32 = np.float32
    bf16 = ml_dtypes.bfloat16
    fp8 = ml_dtypes.float8_e4m3
    x = np.asarray(inputs["x"], f32)

    def wT(W):  # [H, dout, din] -> [H, din, dout]
        return np.ascontiguousarray(np.asarray(W, f32).transpose(0, 2, 1))

    w1t = np.ascontiguousarray(
        np.asarray(inputs["W1"], f32).transpose(2, 0, 1).reshape(4, H * D)
    ).astype(bf16)

    def mid(Wname):
        W = wT(inputs[Wname])  # [H, 256, 256]
        return W.reshape(H, 2, 128, D).transpose(0, 2, 1, 3)  # [H, 128, 2, D]

    # [H, 128, 5(layer), 2(kt), D]
    whead = np.ascontiguousarray(
        np.stack([mid(n) for n in ("W2", "W31", "W32", "W41", "W42")], axis=2)
    ).astype(fp8)

    def bias(bname):
        b = np.asarray(inputs[bname], f32)  # [H, 256]
        return np.ascontiguousarray(b.reshape(H, 2, 128).transpose(2, 0, 1))

    w51T = wT(inputs["W51"])  # [H, 256, 2]
    w52T = wT(inputs["W52"])
    # [128, H, branch, kt, 16], scaled by L5SC for fp8 range
    l5wv = np.zeros((128, H, 2, 2, 32), f32)
    for kt in range(2):
        ksl = slice(kt * 128, (kt + 1) * 128)
        l5wv[:, :, 0, kt, 0:2] = -L5SC * w51T[:, ksl, :].transpose(1, 0, 2)
        l5wv[:, :, 1, kt, 2:4] = L5SC * w52T[:, ksl, :].transpose(1, 0, 2)
    l5wv = np.ascontiguousarray(l5wv).astype(fp8)

    shared = {
        "w1t": w1t,
        "whead": whead,
        "b1": bias("b1"), "b2": bias("b2"), "b31": bias("b31"),
        "b32": bias("b32"), "b41": bias("b41"), "b42": bias("b42"),
        "l5w": l5wv,
        "b51v": np.asarray(inputs["b51"], f32).reshape(1, 2 * H),
        "b52v": np.asarray(inputs["b52"], f32).reshape(1, 2 * H),
        "wt": np.asarray(inputs["wt"], f32).reshape(1, H),
        "mlab": np.asarray(inputs["mean_label"], f32).reshape(1, 2),
        "slab": np.asarray(inputs["std_label"], f32).reshape(1, 2),
        "meanp": np.asarray(inputs["mean"], f32).reshape(1, 4),
        "stdp": np.asarray(inputs["std"], f32).reshape(1, 4),
    }

    in_maps = []
    for c in range(NCORES):
        xs = x[c * BC : (c + 1) * BC]
        m = dict(shared)
        m["xT"] = np.ascontiguousarray(xs.T).astype(bf16)
        m["xg"] = np.ascontiguousarray(xs.reshape(NT, 128, 4).transpose(1, 0, 2))
        in_maps.append(m)
    return in_maps


_NC_CACHE = {}


def get_graph():
    if "nc" not in _NC_CACHE:
        _NC_CACHE["nc"] = build_graph()
    return _NC_CACHE["nc"]


def kernel(**inputs) -> np.ndarray:
    from concourse.bass_utils import run_bass_kernel_spmd

    nc = get_graph()
    in_maps = prep_inputs(inputs)
    res = run_bass_kernel_spmd(nc, in_maps, core_ids=list(range(NCORES)))
    return np.concatenate(
        [np.asarray(res.results[i]["out"], np.float32) for i in range(NCORES)], axis=0
    )

